# revision 1
# baseline (speedup 1.0000x reference)
"""AFNB (Asymmetric Fusion Non-local Block) — distributed Bass kernel for
8 Trainium2 NeuronCores. Self-contained: builds the Bass/Tile graph, shards
the full inputs, runs SPMD via bass_utils.run_bass_kernel_spmd, and gathers
the full output.

Sharding: data-parallel over (batch, row-half) -> 8 shards of 2048 pixels
(batch b = core//2, rows r0 = (core%2)*32 .. +32).

v3 design vs the v1 baseline:
- conv_W and the context half of conv_o are folded on the host:
  wfused = o_scale * (wo[:, :512] @ wW), so the whole context->output path
  is one small matrix W' = wfused @ vp applied to the attention map. This
  eliminates the full-res conv_W and the ctxW half of conv_o.
- conv_k and conv_v run FUSED in px-major orientation (lhsT = xlow chunk,
  rhs = [wk|wv]) into a single [128px, 512] PSUM group, so PSP pooling
  needs no PE transposes at all: pooled kp/vp come straight from
  [px, c]-major SBUF tiles x the P matrix. The k-BN scale is folded into
  wk on the host; the k-BN bias is added with a rank-1 ones-matmul into
  the same PSUM group (pre-ReLU); the v bias is added post-pooling as
  bv x colsum(P_local).
- ONE pairwise AllGather ships kp+vp partials (the collective-cores
  resource serializes collectives, so a single exchange beats two). Its
  latency is hidden under conv_q and all 16 high-half conv_o tiles, whose
  results are parked in SBUF (bf16) and re-added at the end with
  scalar_tensor_tensor (split across DVE and Pool engines).

All matmul operands are bf16 (PSUM accumulation fp32). Softmax is computed
in the transposed [bins, px] layout: logits are bounded (|sim|/16 < ~5), so
exp() needs no max-subtraction; partition-axis sum/broadcast use tiny
ones-matmuls on the TensorEngine.
"""
import numpy as np
import ml_dtypes

import concourse.bass as bass
import concourse.mybir as mybir
import concourse.tile as tile
from concourse.bacc import Bacc
from concourse import bass_utils

F32 = mybir.dt.float32
BF16 = mybir.dt.bfloat16
AF = mybir.ActivationFunctionType

N_CORES = 8
B, H, W = 4, 64, 64
LOW_C, HIGH_C, KEY_C, VAL_C, OUT_C = 1024, 512, 256, 256, 512
PSP_SIZES = (1, 3, 6, 8)
NBINS = 110
PX = 2048
NCHUNK = 16  # px chunks of 128
EPS = 1e-5

PACK_N = 4 * NBINS * 128  # kp (2x[128,110]) + vp (2x[128,110]) partials


def build_kernel(n_rep: int = 1):
    nc = Bacc("TRN2", target_bir_lowering=False, num_devices=N_CORES)

    xlow = nc.dram_tensor("xlow", [128, 8, PX], BF16, kind="ExternalInput")
    xhigh = nc.dram_tensor("xhigh", [128, 4, PX], BF16, kind="ExternalInput")
    wkvT = nc.dram_tensor("wkvT", [128, 8, 512], BF16, kind="ExternalInput")
    wqT = nc.dram_tensor("wqT", [128, 4, KEY_C], BF16, kind="ExternalInput")
    woT = nc.dram_tensor("woT", [128, 4, OUT_C], BF16, kind="ExternalInput")
    wfT = nc.dram_tensor("wfT", [128, 2, OUT_C], BF16, kind="ExternalInput")
    Pmat = nc.dram_tensor("Pmat", [128, 16, NBINS], BF16, kind="ExternalInput")
    ident = nc.dram_tensor("ident", [128, 128], BF16, kind="ExternalInput")
    kbrow = nc.dram_tensor("kbrow", [1, KEY_C], BF16, kind="ExternalInput")
    bvrow = nc.dram_tensor("bvrow", [1, VAL_C], BF16, kind="ExternalInput")
    csum = nc.dram_tensor("csum", [1, NBINS], BF16, kind="ExternalInput")
    qvec = nc.dram_tensor("qvec", [128, 4], F32, kind="ExternalInput")
    obias = nc.dram_tensor("obias", [128, 4], F32, kind="ExternalInput")
    out = nc.dram_tensor("out", [128, 4, PX], F32, kind="ExternalOutput")

    with tile.TileContext(nc) as tc:
        with (
            tc.tile_pool(name="const", bufs=1) as cpool,
            tc.tile_pool(name="xin", bufs=2) as xpool,
            tc.tile_pool(name="work", bufs=2) as wpool,
            tc.tile_pool(name="vecs", bufs=4) as vpool,
            tc.tile_pool(name="outp", bufs=4) as opool,
            tc.tile_pool(name="psum", bufs=2, space="PSUM") as pspool,
            tc.tile_pool(name="dram", bufs=1, space="DRAM") as dpool,
        ):
            wkv_sb = cpool.tile([128, 8, 512], BF16)
            wq_sb = cpool.tile([128, 4, KEY_C], BF16)
            wo_sb = cpool.tile([128, 4, OUT_C], BF16)
            wf_sb = cpool.tile([128, 2, OUT_C], BF16)
            P_sb = cpool.tile([128, 16, NBINS], BF16)
            id_sb = cpool.tile([128, 128], BF16)
            kb_sb = cpool.tile([1, KEY_C], BF16)
            bv_sb = cpool.tile([1, VAL_C], BF16)
            cs_sb = cpool.tile([1, NBINS], BF16)
            qv_sb = cpool.tile([128, 4], F32)
            ob_sb = cpool.tile([128, 4], F32)
            xh_sb = cpool.tile([128, 4, PX], BF16)
            q_sb = cpool.tile([128, 2, PX], BF16, tag="qsb")
            high_sb = cpool.tile([128, 16, 512], BF16, tag="hsb")
            WT_sb = cpool.tile([NBINS, OUT_C], BF16, tag="WT")

            for rep in range(n_rep):
                if rep > 0:
                    tc.strict_bb_all_engine_barrier()
                # ACT warmup: hoist the act-table load into the DMA phase
                warm = vpool.tile([128, 1], F32, tag="warm")
                nc.vector.memset(warm[:], 0.0)
                warm2 = vpool.tile([128, 1], F32, tag="warm2")
                nc.scalar.activation(warm2[:], warm[:], AF.Relu)
                nc.scalar.activation(warm2[:], warm[:], AF.Exp)
                ones_col = cpool.tile([NBINS, 1], BF16, tag="ones_col")
                nc.vector.memset(ones_col[:], 1.0)
                ones_row = cpool.tile([1, NBINS], BF16, tag="ones_row")
                nc.vector.memset(ones_row[:], 1.0)
                ones1 = cpool.tile([1, 128], BF16, tag="ones1")
                nc.vector.memset(ones1[:], 1.0)

                # stage-1-critical loads first; PE warmup matmuls fill the
                # DMA wait and ramp the PE p-state
                nc.sync.dma_start(id_sb[:], ident[:])
                warm_ps = pspool.tile([128, 512], F32, tag="convps")
                for _w in range(52):
                    nc.tensor.matmul(warm_ps[:, 0:128], id_sb[:], id_sb[:],
                                     start=True, stop=True)
                xl_blks = [xpool.tile([128, 8, 512], BF16, tag="xlblk",
                                      bufs=4, name=f"xlblk{n}")
                           for n in range(4)]
                nc.sync.dma_start(kb_sb[:], kbrow[:])
                nc.sync.dma_start(wkv_sb[:], wkvT[:])
                nc.sync.dma_start(xl_blks[0][:, 0:4, :],
                                  xlow[:, 0:4, 0:512])
                nc.sync.dma_start(xl_blks[0][:, 4:8, :],
                                  xlow[:, 4:8, 0:512])
                nc.sync.dma_start(P_sb[:], Pmat[:])
                for n in range(1, 4):
                    nc.sync.dma_start(xl_blks[n][:, 0:4, :],
                                      xlow[:, 0:4, 512 * n:512 * n + 512])
                    nc.sync.dma_start(xl_blks[n][:, 4:8, :],
                                      xlow[:, 4:8, 512 * n:512 * n + 512])
                # secondary stream
                nc.scalar.dma_start(bv_sb[:], bvrow[:])
                nc.scalar.dma_start(cs_sb[:], csum[:])
                nc.scalar.dma_start(wq_sb[:], wqT[:])
                nc.scalar.dma_start(qv_sb[:], qvec[:])
                nc.scalar.dma_start(xh_sb[:], xhigh[:])
                nc.gpsimd.dma_start(wf_sb[:], wfT[:])
                nc.gpsimd.dma_start(wo_sb[:], woT[:])
                nc.gpsimd.dma_start(ob_sb[:], obias[:])

                # long-lived PSUM accumulators: kp (2 groups) + vp (2 groups)
                # in one bank
                kvp_ps = pspool.tile([128, 4, NBINS], F32, bufs=1, tag="kvp")

                # ==== STAGE 1: fused conv_k|conv_v px-major + PSP pooling ====
                # software-pipelined: the pool matmuls of chunk c-1 issue
                # between the conv matmuls of chunk c, so the PE never waits
                # on the ACT/DVE chunk post-processing
                ksbs, vsbs = [None] * NCHUNK, [None] * NCHUNK

                def pool_mms(c):
                    for m in range(2):
                        nc.tensor.matmul(
                            kvp_ps[:, m, :], ksbs[c][:, m * 128:(m + 1) * 128],
                            P_sb[:, c, :],
                            start=(c == 0), stop=(c == NCHUNK - 1),
                            skip_group_check=True)
                    for m in range(2):
                        nc.tensor.matmul(
                            kvp_ps[:, 2 + m, :],
                            vsbs[c][:, m * 128:(m + 1) * 128],
                            P_sb[:, c, :],
                            start=False, stop=(c == NCHUNK - 1),
                            skip_group_check=True)

                # vp bias bv x colsum(P_local) opens the vp groups, off the
                # critical pack chain
                for m in range(2):
                    nc.tensor.matmul(
                        kvp_ps[:, 2 + m, :], bv_sb[:, m * 128:(m + 1) * 128],
                        cs_sb[:], start=True, stop=False,
                        skip_group_check=True)

                for c in range(NCHUNK):
                    n, cc = c // 4, c % 4
                    kvps = pspool.tile([128, 512], F32, tag="kvconv")
                    for kc in range(8):
                        nc.tensor.matmul(
                            kvps[:],
                            xl_blks[n][:, kc, cc * 128:cc * 128 + 128],
                            wkv_sb[:, kc, :],
                            start=(kc == 0), stop=False, skip_group_check=True)
                        if kc == 4 and c > 0:
                            pool_mms(c - 1)
                    # k-half BN bias as rank-1 ones-matmul, pre-ReLU
                    nc.tensor.matmul(kvps[:, 0:KEY_C], ones1[:], kb_sb[:],
                                     start=False, stop=True,
                                     skip_group_check=True)
                    ksb = wpool.tile([128, KEY_C], BF16, tag="ksb", bufs=4)
                    nc.scalar.activation(ksb[:], kvps[:, 0:KEY_C], AF.Relu)
                    vsb = wpool.tile([128, VAL_C], BF16, tag="vsb", bufs=4)
                    nc.vector.tensor_copy(vsb[:], kvps[:, KEY_C:512])
                    ksbs[c], vsbs[c] = ksb, vsb
                pool_mms(NCHUNK - 1)

                # pack partials; single pairwise AllGather
                kv_h = wpool.tile([128, 4 * NBINS], BF16, tag="kv_h")
                nc.vector.tensor_copy(kv_h[:], kvp_ps[:])
                cc_in = dpool.tile([PACK_N], BF16, tag=f"ccin_{rep}")
                cc_out = dpool.tile([2, PACK_N], BF16, tag=f"ccout_{rep}")
                nc.scalar.dma_start(
                    cc_in[:].rearrange("(p f) -> p f", p=128), kv_h[:])
                nc.gpsimd.collective_compute(
                    "AllGather", mybir.AluOpType.bypass,
                    replica_groups=[[0, 1], [2, 3], [4, 5], [6, 7]],
                    ins=[cc_in[:].opt()], outs=[cc_out[:].opt()])

                # conv_q — independent of the collective; covers AG latency
                for n in range(4):
                    for m in range(2):
                        qps = pspool.tile([128, 512], F32, tag="convps")
                        for kc in range(4):
                            nc.tensor.matmul(
                                qps[:], wq_sb[:, kc, m * 128:(m + 1) * 128],
                                xh_sb[:, kc, n * 512:(n + 1) * 512],
                                start=(kc == 0), stop=(kc == 3))
                        nc.scalar.activation(
                            q_sb[:, m, n * 512:(n + 1) * 512], qps[:], AF.Relu,
                            bias=qv_sb[:, 2 + m:3 + m], scale=qv_sb[:, m:m + 1])

                # high-half conv_o for ALL blocks — also collective cover;
                # park bf16 results in SBUF (copies split DVE/Pool)
                # (copies split ACT/DVE; Pool cannot read PSUM, and its queue
                # is blocked behind the collective anyway)
                for n in range(4):
                    for om in range(4):
                        hps = pspool.tile(
                            [128, 512], F32,
                            tag="convps" if om % 2 == 0 else "kvconv")
                        for kc in range(4):
                            nc.tensor.matmul(
                                hps[:], wo_sb[:, kc, om * 128:(om + 1) * 128],
                                xh_sb[:, kc, n * 512:(n + 1) * 512],
                                start=(kc == 0), stop=(kc == 3))
                        eng = nc.scalar if om % 2 == 0 else nc.vector
                        if eng is nc.scalar:
                            nc.scalar.copy(high_sb[:, n * 4 + om, :], hps[:])
                        else:
                            nc.vector.tensor_copy(high_sb[:, n * 4 + om, :],
                                                  hps[:])

                # PE fill through any remaining collective latency: keeps the
                # p-state hot so stage 2 starts at full clock
                for _w in range(24):
                    nc.tensor.matmul(warm_ps[:, 0:128], id_sb[:], id_sb[:],
                                     start=True, stop=True)

                # combine gathered partials: kp_b [256,110], vp_b [2x128,110]
                kv_all = wpool.tile([128, 2, 4 * NBINS], BF16, tag="kv_all")
                nc.scalar.dma_start(
                    kv_all[:],
                    cc_out[:, :].rearrange("r (p f) -> p r f", p=128))
                vp_b = wpool.tile([128, 2, NBINS], BF16, tag="vp_b")
                kp_b = wpool.tile([128, 2 * NBINS], BF16, tag="kp_b")
                with nc.allow_low_precision(reason="pair partial sum in bf16"):
                    nc.vector.tensor_add(kp_b[:], kv_all[:, 0, 0:2 * NBINS],
                                         kv_all[:, 1, 0:2 * NBINS])
                    nc.gpsimd.tensor_add(vp_b[:],
                                         kv_all[:, 0, 2 * NBINS:4 * NBINS],
                                         kv_all[:, 1, 2 * NBINS:4 * NBINS])

                # ======== STAGE 2: attention + fused output ========
                # pipelined 2 blocks deep; W'T = vp^T @ wfused^T computed
                # while ACT runs exp(block 0)
                sim_ps = [None] * 4
                attnUs = [None] * 4

                def sim_block(n):
                    sps = pspool.tile([NBINS, 512], F32, tag="convps",
                                      name=f"sps{n}")
                    for m in range(2):
                        nc.tensor.matmul(
                            sps[:], kp_b[:, m * NBINS:(m + 1) * NBINS],
                            q_sb[:, m, n * 512:n * 512 + 512],
                            start=(m == 0), stop=(m == 1))
                    # softmax over partitions: logits bounded -> no max-sub
                    attnU = wpool.tile([NBINS, 512], BF16, tag="attnU", bufs=2,
                                       name=f"attnU{n}")
                    nc.scalar.activation(attnU[:], sps[:], AF.Exp,
                                         bias=0.0, scale=0.0625)
                    sim_ps[n], attnUs[n] = sps, attnU

                sim_block(0)
                WT_ps = pspool.tile([NBINS, OUT_C], F32, tag="convps")
                for m in range(2):
                    nc.tensor.matmul(WT_ps[:], vp_b[:, m, :], wf_sb[:, m, :],
                                     start=(m == 0), stop=(m == 1))
                nc.vector.tensor_copy(WT_sb[:], WT_ps[:])
                sim_block(1)

                for n in range(4):
                    px0 = n * 512
                    attnU = attnUs[n]
                    # kvp bank is dead after the partial pack — reuse it so
                    # consecutive blocks' softmax chains don't share a slot
                    ps_row = pspool.tile([1, 512], F32, tag="kvp", bufs=1)
                    nc.tensor.matmul(ps_row[:], ones_col[:], attnU[:],
                                     start=True, stop=True)
                    rs_sb = vpool.tile([1, 512], BF16, tag="rs_sb")
                    with nc.allow_low_precision(reason="softmax 1/sum in bf16"):
                        nc.vector.reciprocal(rs_sb[:], ps_row[:])
                    ps_bc = pspool.tile([NBINS, 512], F32, tag="kps", bufs=1)
                    nc.tensor.matmul(ps_bc[:], ones_row[:], rs_sb[:],
                                     start=True, stop=True)
                    attnT = wpool.tile([NBINS, 512], BF16, tag="attnT", bufs=2)
                    nc.vector.tensor_mul(attnT[:], attnU[:], ps_bc[:])

                    # out[oc, px] = W'T.T @ attnT (+ parked high + obias);
                    # Pool cannot read PSUM, so om-odd tiles go
                    # ACT (psum+bias) -> Pool (SBUF add)
                    for om in range(4):
                        ops = pspool.tile([128, 512], F32, tag="outps")
                        nc.tensor.matmul(
                            ops[:], WT_sb[:, om * 128:(om + 1) * 128],
                            attnT[:], start=True, stop=True)
                        osb = opool.tile([128, 512], F32, tag="osb")
                        if om % 2 == 0:
                            nc.vector.scalar_tensor_tensor(
                                osb[:], ops[:], ob_sb[:, om:om + 1],
                                high_sb[:, n * 4 + om, :],
                                op0=mybir.AluOpType.add,
                                op1=mybir.AluOpType.add)
                        else:
                            ot = opool.tile([128, 512], F32, tag="ot")
                            nc.scalar.activation(
                                ot[:], ops[:], AF.Identity,
                                bias=ob_sb[:, om:om + 1], scale=1.0)
                            nc.gpsimd.tensor_add(
                                osb[:], ot[:], high_sb[:, n * 4 + om, :])
                        deng = nc.sync if om % 2 == 0 else nc.scalar
                        deng.dma_start(
                            out[:, om, px0:px0 + 512], osb[:])
                    if n + 2 < 4:
                        sim_block(n + 2)

    nc.finalize()
    return nc


# ======================= host-side data prep =======================

def _pool_matrix_rows(r0, nrows):
    P = np.zeros((nrows * W, NBINS), np.float32)
    col = 0
    for s in PSP_SIZES:
        hs = np.floor(np.arange(s) * H / s).astype(int)
        he = np.ceil((np.arange(s) + 1) * H / s).astype(int)
        ws = np.floor(np.arange(s) * W / s).astype(int)
        we = np.ceil((np.arange(s) + 1) * W / s).astype(int)
        for bh in range(s):
            for bw in range(s):
                area = (he[bh] - hs[bh]) * (we[bw] - ws[bw])
                m = np.zeros((nrows, W), np.float32)
                lo = max(hs[bh] - r0, 0)
                hi = min(he[bh] - r0, nrows)
                if lo < hi:
                    m[lo:hi, ws[bw]:we[bw]] = 1.0 / area
                P[:, col] = m.reshape(-1)
                col += 1
    return P


def _fold_bn(gamma, beta, mean, var, conv_bias):
    inv = np.asarray(gamma, np.float64) / np.sqrt(np.asarray(var, np.float64) + EPS)
    shift = inv * np.asarray(conv_bias, np.float64) + np.asarray(beta, np.float64) \
        - np.asarray(mean, np.float64) * inv
    return inv.astype(np.float32), shift.astype(np.float32)


def _as_kxm(wt, n_k):
    M, K = wt.shape
    r = np.asarray(wt, np.float32).T.reshape(n_k, 128, M).transpose(1, 0, 2)
    return np.ascontiguousarray(r.astype(ml_dtypes.bfloat16))


def prep_in_maps(inputs):
    bf = ml_dtypes.bfloat16
    lf = np.asarray(inputs['low_feats'], np.float32)
    hf = np.asarray(inputs['high_feats'], np.float32)

    ks, kb = _fold_bn(inputs['k_gamma'], inputs['k_beta'], inputs['k_mean'],
                      inputs['k_var'], inputs['bk'])
    qs, qb = _fold_bn(inputs['q_gamma'], inputs['q_beta'], inputs['q_mean'],
                      inputs['q_var'], inputs['bq'])
    os_, ob = _fold_bn(inputs['o_gamma'], inputs['o_beta'], inputs['o_mean'],
                       inputs['o_var'], inputs['bo'])

    # fused conv_k|conv_v weights (k-BN scale folded into wk); the k bias
    # is added on-device as a rank-1 matmul
    wk_f = np.asarray(inputs['wk'], np.float32) * ks[:, None]
    wkv = np.concatenate([wk_f, np.asarray(inputs['wv'], np.float32)], axis=0)
    wkv_h = _as_kxm(wkv, 8)
    kbrow = np.asarray(kb, np.float32).reshape(1, KEY_C).astype(bf)
    bvrow = np.asarray(inputs['bv'], np.float32).reshape(1, VAL_C).astype(bf)

    wq_h = _as_kxm(np.asarray(inputs['wq']), 4)
    qvec = np.stack([qs[0:128], qs[128:256], qb[0:128], qb[128:256]],
                    1).astype(np.float32)

    # fused context path: W-conv folded into conv_o's ctx half + o-BN scale
    wo = np.asarray(inputs['wo'], np.float32)
    wW = np.asarray(inputs['wW'], np.float32)
    bW = np.asarray(inputs['bW'], np.float32)
    wfused = os_[:, None] * (wo[:, :OUT_C] @ wW)          # [512, 256]
    wo_high = os_[:, None] * wo[:, OUT_C:]                # [512, 512]
    obias_t = ob + os_ * (wo[:, :OUT_C] @ bW)             # [512]
    wf_h = _as_kxm(wfused, 2)
    wo_h = _as_kxm(wo_high, 4)
    obias = np.stack([obias_t[i * 128:(i + 1) * 128] for i in range(4)],
                     1).astype(np.float32)

    ident = np.eye(128, dtype=bf)

    in_maps = []
    for core in range(N_CORES):
        b, half = core // 2, core % 2
        r0 = half * 32
        xl = lf[b, :, r0:r0 + 32, :].reshape(LOW_C, PX)
        xl_h = np.ascontiguousarray(
            xl.reshape(8, 128, PX).transpose(1, 0, 2).astype(bf))
        xh = hf[b, :, r0:r0 + 32, :].reshape(HIGH_C, PX)
        xh_h = np.ascontiguousarray(
            xh.reshape(4, 128, PX).transpose(1, 0, 2).astype(bf))
        P = _pool_matrix_rows(r0, 32)
        P_h = np.ascontiguousarray(
            P.reshape(16, 128, NBINS).transpose(1, 0, 2).astype(bf))
        csum = P.sum(0).reshape(1, NBINS).astype(bf)
        in_maps.append({
            "xlow": xl_h, "xhigh": xh_h,
            "wkvT": wkv_h, "wqT": wq_h, "woT": wo_h, "wfT": wf_h,
            "Pmat": P_h, "ident": ident,
            "kbrow": kbrow, "bvrow": bvrow, "csum": csum,
            "qvec": qvec, "obias": obias,
        })
    return in_maps


def assemble_output(core_outs):
    full = np.zeros((B, OUT_C, H, W), np.float32)
    for core in range(N_CORES):
        b, half = core // 2, core % 2
        r0 = half * 32
        arr = np.asarray(core_outs[core]["out"])
        ch_px = arr.reshape(128, 4, 32, W).transpose(1, 0, 2, 3).reshape(
            OUT_C, 32, W)
        full[b, :, r0:r0 + 32, :] = ch_px
    return full


_CACHED_NC = None


def kernel(**inputs) -> np.ndarray:
    global _CACHED_NC
    if _CACHED_NC is None:
        _CACHED_NC = build_kernel(n_rep=1)
    in_maps = prep_in_maps(inputs)
    res = bass_utils.run_bass_kernel_spmd(
        _CACHED_NC, in_maps, core_ids=list(range(N_CORES)))
    return assemble_output(res.results)



# revision 15
# speedup vs baseline: 1.4661x; 1.4661x over previous
"""AFNB (Asymmetric Fusion Non-local Block) — distributed Bass kernel for
8 Trainium2 NeuronCores. Self-contained: builds the Bass/Tile graph, shards
the full inputs, runs SPMD via bass_utils.run_bass_kernel_spmd, and gathers
the full output.

Sharding: data-parallel over (batch, row-half) -> 8 shards of 2048 pixels
(batch b = core//2, rows r0 = (core%2)*32 .. +32).

v4 design (vs the v3 baseline at ~90us sim / ~93us HW):
- The value path exploits linearity of PSP pooling: PSP(conv_v(x)) =
  wv @ PSP(x). The input xlow is pooled directly on the PE from a px-major
  copy (xlt, shipped as an extra input; host prep is free), then vp =
  wvT @ pooled_low costs 1.8k cycles. This removes the full-res conv_v
  (-33k PE cycles) at the cost of 14k pooling cycles that double as
  PE-ramp work.
- The kp|vp AllGather (fixed ~15us launch + transfer) is issued right
  after stage 1 (~27us, vs ~44us in v3) and hidden under conv_q and the
  px-major conv_o-high, whose 49k cycles of PE work almost exactly cover
  the ~20.6us collective.
- Softmax normalization is deferred past the output matmul: the context
  matmuls consume the UNNORMALIZED exp map in px-major orientation
  (lhsT = attnU chunk), so out[px, oc]*(1/s[px]) is a per-partition scale
  applied by ACT/DVE in the final combine, where the high half (+ o-bias,
  folded during the cover phase from a host-prebroadcast row) is added.
  This removes the per-block broadcast matmuls and the serial
  sum->recip->mul chain in front of the context matmuls.

All matmul operands are bf16 (PSUM accumulation fp32). exp() needs no
max-subtraction: |sim|/16 < ~5.
"""
import numpy as np
import ml_dtypes

import concourse.bass as bass
import concourse.mybir as mybir
import concourse.tile as tile
from concourse.bacc import Bacc
from concourse import bass_utils

F32 = mybir.dt.float32
BF16 = mybir.dt.bfloat16
AF = mybir.ActivationFunctionType
ALU = mybir.AluOpType

N_CORES = 8
B, H, W = 4, 64, 64
LOW_C, HIGH_C, KEY_C, VAL_C, OUT_C = 1024, 512, 256, 256, 512
PSP_SIZES = (1, 3, 6, 8)
NBINS = 110
PX = 2048
NCHUNK = 16  # px chunks of 128
EPS = 1e-5

PACK_N = 4 * NBINS * 128  # kp (2x[128,110]) + vp (2x[128,110]) partials


def build_kernel(n_rep: int = 1):
    nc = Bacc("TRN2", target_bir_lowering=False, num_devices=N_CORES)

    xlow = nc.dram_tensor("xlow", [128, 8, PX], BF16, kind="ExternalInput")
    xlt = nc.dram_tensor("xlt", [128, NCHUNK, LOW_C], BF16,
                         kind="ExternalInput")
    xhigh = nc.dram_tensor("xhigh", [128, 4, PX], BF16, kind="ExternalInput")
    wkT = nc.dram_tensor("wkT", [128, 8, KEY_C], BF16, kind="ExternalInput")
    wvT = nc.dram_tensor("wvT", [128, 8, VAL_C], BF16, kind="ExternalInput")
    wqT = nc.dram_tensor("wqT", [128, 4, KEY_C], BF16, kind="ExternalInput")
    woT = nc.dram_tensor("woT", [128, 4, OUT_C], BF16, kind="ExternalInput")
    wfT = nc.dram_tensor("wfT", [128, 2, OUT_C], BF16, kind="ExternalInput")
    Pmat = nc.dram_tensor("Pmat", [128, 16, NBINS], BF16, kind="ExternalInput")
    kbbc = nc.dram_tensor("kbbc", [128, KEY_C], F32, kind="ExternalInput")
    bvrow = nc.dram_tensor("bvrow", [1, VAL_C], BF16, kind="ExternalInput")
    csum = nc.dram_tensor("csum", [1, NBINS], BF16, kind="ExternalInput")
    qvec = nc.dram_tensor("qvec", [128, 4], F32, kind="ExternalInput")
    obrow = nc.dram_tensor("obrow", [128, OUT_C], F32, kind="ExternalInput")
    out = nc.dram_tensor("out", [128, NCHUNK, OUT_C], F32,
                         kind="ExternalOutput")

    with tile.TileContext(nc) as tc:
        with (
            tc.tile_pool(name="const", bufs=1) as cpool,
            tc.tile_pool(name="xin", bufs=2) as xpool,
            tc.tile_pool(name="work", bufs=2) as wpool,
            tc.tile_pool(name="vecs", bufs=4) as vpool,
            tc.tile_pool(name="outp", bufs=4) as opool,
            tc.tile_pool(name="psum", bufs=2, space="PSUM") as pspool,
            tc.tile_pool(name="dram", bufs=1, space="DRAM") as dpool,
        ):
            wk_sb = cpool.tile([128, 8, KEY_C], BF16)
            wv_sb = cpool.tile([128, 8, VAL_C], BF16)
            wq_sb = cpool.tile([128, 4, KEY_C], BF16)
            wo_sb = cpool.tile([128, 4, OUT_C], BF16)
            wf_sb = cpool.tile([128, 2, OUT_C], BF16)
            P_sb = cpool.tile([128, 16, NBINS], BF16)
            kb_sb = cpool.tile([128, KEY_C], F32)
            bv_sb = cpool.tile([1, VAL_C], BF16)
            cs_sb = cpool.tile([1, NBINS], BF16)
            qv_sb = cpool.tile([128, 4], F32)
            ob_sb = cpool.tile([128, OUT_C], F32)
            xh_sb = cpool.tile([128, 4, PX], BF16, tag="xhsb")
            q_sb = cpool.tile([128, 2, PX], BF16, tag="qsb")
            high2 = cpool.tile([128, NCHUNK, OUT_C], BF16, tag="hsb")
            WT_sb = cpool.tile([NBINS, OUT_C], BF16, tag="WT")
            pl_sb = cpool.tile([128, 8, NBINS], BF16, tag="plsb")

            for rep in range(n_rep):
                if rep > 0:
                    tc.strict_bb_all_engine_barrier()
                # warm tiles via memset: no DMA dependency for PE ramp work
                warm = vpool.tile([128, 128], BF16, tag="warm")
                nc.vector.memset(warm[:], 0.0)
                warm2 = vpool.tile([128, 1], F32, tag="warm2")
                # ACT warmup: hoist the act-table load off the critical path
                nc.scalar.activation(warm2[:], warm[:, 0:1], AF.Relu)
                nc.scalar.activation(warm2[:], warm[:, 0:1], AF.Exp)
                ones_col = cpool.tile([NBINS, 1], BF16, tag="ones_col")
                nc.vector.memset(ones_col[:], 1.0)

                warm_ps = pspool.tile([128, 512], F32, tag="convps")
                for _w in range(20):
                    nc.tensor.matmul(warm_ps[:, 0:128], warm[:], warm[:],
                                     start=True, stop=True)

                # ---- DMA schedule ----
                # ACT queue: small rows only (ACT must stay free for the
                # k-relus that gate the kpool matmuls)
                xlt_blks = [None] * NCHUNK
                nc.scalar.dma_start(kb_sb[:], kbbc[:])
                nc.scalar.dma_start(bv_sb[:], bvrow[:])
                nc.scalar.dma_start(cs_sb[:], csum[:])
                for c in range(NCHUNK):
                    xlt_blks[c] = xpool.tile([128, LOW_C], BF16, tag="xlt",
                                             bufs=16, name=f"xlt{c}")
                # Pool queue: P first (gates lowpool), all xlt chunks, then
                # cover weights
                nc.gpsimd.dma_start(P_sb[:], Pmat[:])
                for c in range(NCHUNK):
                    nc.gpsimd.dma_start(xlt_blks[c][:], xlt[:, c, :])
                nc.gpsimd.dma_start(wq_sb[:], wqT[:])
                nc.gpsimd.dma_start(qv_sb[:], qvec[:])
                nc.gpsimd.dma_start(wo_sb[:], woT[:])
                nc.gpsimd.dma_start(wf_sb[:], wfT[:])
                nc.gpsimd.dma_start(ob_sb[:], obrow[:])
                # SP queue: conv_k weights + ch-major xlow stream + xh
                xl_blks = [xpool.tile([128, 8, 512], BF16, tag="xlblk",
                                      bufs=4, name=f"xlblk{n}")
                           for n in range(4)]
                nc.sync.dma_start(wk_sb[:], wkT[:])
                for n in range(4):
                    nc.sync.dma_start(xl_blks[n][:, 0:4, :],
                                      xlow[:, 0:4, 512 * n:512 * n + 512])
                    nc.sync.dma_start(xl_blks[n][:, 4:8, :],
                                      xlow[:, 4:8, 512 * n:512 * n + 512])
                nc.sync.dma_start(wv_sb[:], wvT[:])
                nc.sync.dma_start(xh_sb[:], xhigh[:])

                # ==== STAGE 1 ====
                # long-lived PSUM accumulators:
                #   pl_ps: pooled xlow, 8 groups of [128, 110]
                #   kvp_ps: kp (2 groups) + vp (2 groups) in one bank
                # groups padded to 128 cols so no [*,110] group straddles
                # a PSUM bank boundary
                pl_ps = pspool.tile([128, 8, 128], F32, bufs=1, tag="plps")
                kvp_ps = pspool.tile([128, 4, NBINS], F32, bufs=1, tag="kvp")

                # vp bias bv x colsum(P_local) opens the vp groups
                for m in range(2):
                    nc.tensor.matmul(
                        kvp_ps[:, 2 + m, :], bv_sb[:, m * 128:(m + 1) * 128],
                        cs_sb[:], start=True, stop=False,
                        skip_group_check=True)

                def lowpool_mms(c):
                    for b in range(8):
                        nc.tensor.matmul(
                            pl_ps[:, b, 0:NBINS],
                            xlt_blks[c][:, b * 128:(b + 1) * 128],
                            P_sb[:, c, :],
                            start=(c == 0), stop=(c == NCHUNK - 1),
                            skip_group_check=True)

                # pool the first chunks while xlow/wk stream in
                for c in range(6):
                    lowpool_mms(c)

                # fused conv_k px-major + PSP pooling of relu(k); the pool
                # matmuls of chunk c-1 issue between the conv matmuls of
                # chunk c so the PE never waits on the ACT post-processing
                ksbs = [None] * NCHUNK

                def kpool_mms(c):
                    for m in range(2):
                        nc.tensor.matmul(
                            kvp_ps[:, m, :],
                            ksbs[c][:, m * 128:(m + 1) * 128],
                            P_sb[:, c, :],
                            start=(c == 0), stop=(c == NCHUNK - 1),
                            skip_group_check=True)

                lp_next = 6
                for c in range(NCHUNK):
                    n, cc = c // 4, c % 4
                    kps = pspool.tile([128, 512], F32, tag="convps",
                                      bufs=2)
                    for kc in range(8):
                        nc.tensor.matmul(
                            kps[:, 0:KEY_C],
                            xl_blks[n][:, kc, cc * 128:cc * 128 + 128],
                            wk_sb[:, kc, :],
                            start=(kc == 0), stop=(kc == 7),
                            skip_group_check=True)
                        if kc == 4 and c > 0:
                            kpool_mms(c - 1)
                    # k-BN bias (prebroadcast row) on DVE, then ReLU on ACT;
                    # keeps the bias matmul off the pre-collective PE path
                    kraw = wpool.tile([128, KEY_C], BF16, tag="kraw", bufs=4)
                    nc.vector.tensor_add(kraw[:], kps[:, 0:KEY_C], kb_sb[:])
                    ksb = wpool.tile([128, KEY_C], BF16, tag="ksb", bufs=4)
                    nc.scalar.activation(ksb[:], kraw[:], AF.Relu)
                    ksbs[c] = ksb
                    if lp_next < NCHUNK:
                        lowpool_mms(lp_next)
                        lp_next += 1
                    if lp_next == NCHUNK:
                        # all pooled-low groups closed: copy out for vp
                        for b in range(8):
                            nc.vector.tensor_copy(pl_sb[:, b, :],
                                                  pl_ps[:, b, 0:NBINS])
                        lp_next += 1
                    if c == 13:
                        # vp = wvT @ pooled_low onto the bias groups; off the
                        # pack gate (kpool 15) critical path
                        for b in range(8):
                            for m in range(2):
                                nc.tensor.matmul(
                                    kvp_ps[:, 2 + m, :],
                                    wv_sb[:, b, m * 128:(m + 1) * 128],
                                    pl_sb[:, b, :],
                                    start=False, stop=(b == 7),
                                    skip_group_check=True)
                kpool_mms(NCHUNK - 1)

                # pack partials; single pairwise AllGather
                kv_h = wpool.tile([128, 4 * NBINS], BF16, tag="kv_h")
                nc.vector.tensor_copy(kv_h[:], kvp_ps[:])
                cc_in = dpool.tile([PACK_N], BF16, tag=f"ccin_{rep}")
                cc_out = dpool.tile([2, PACK_N], BF16, tag=f"ccout_{rep}")
                nc.sync.dma_start(
                    cc_in[:].rearrange("(p f) -> p f", p=128), kv_h[:])
                nc.gpsimd.collective_compute(
                    "AllGather", mybir.AluOpType.bypass,
                    replica_groups=[[0, 1], [2, 3], [4, 5], [6, 7]],
                    ins=[cc_in[:].opt()], outs=[cc_out[:].opt()])

                # ==== COVER PHASE (hides the collective) ====
                # conv_q — k-major: lhsT = wq, rhs = xh
                for n in range(4):
                    for m in range(2):
                        qps = pspool.tile([128, 512], F32, tag="convps")
                        for kc in range(4):
                            nc.tensor.matmul(
                                qps[:], wq_sb[:, kc, m * 128:(m + 1) * 128],
                                xh_sb[:, kc, n * 512:(n + 1) * 512],
                                start=(kc == 0), stop=(kc == 3))
                        nc.scalar.activation(
                            q_sb[:, m, n * 512:(n + 1) * 512], qps[:], AF.Relu,
                            bias=qv_sb[:, 2 + m:3 + m], scale=qv_sb[:, m:m + 1])

                # conv_o high half, px-major: lhsT = xh chunk, rhs = wo.
                # high2 = conv + o-bias (prebroadcast row), parked bf16.
                for c in range(NCHUNK):
                    hps = pspool.tile([128, OUT_C], F32, tag="oconv", bufs=3)
                    for kc in range(4):
                        nc.tensor.matmul(
                            hps[:], xh_sb[:, kc, c * 128:c * 128 + 128],
                            wo_sb[:, kc, :],
                            start=(kc == 0), stop=(kc == 3))
                    nc.vector.tensor_add(high2[:, c, :], hps[:], ob_sb[:])

                # PE keep-warm fill through the collective window: the PE
                # would otherwise idle ~7us here and drop its p-state, which
                # would halve the clock for the first ~3us of stage 2
                warmf = pspool.tile([128, 512], F32, tag="convps",
                                    name="warmf")
                for _w in range(110):
                    nc.tensor.matmul(warmf[:, 0:128], warm[:], warm[:],
                                     start=True, stop=True)

                # combine gathered partials: kp half lands first (on SP) so
                # the sim matmuls can start while the vp half (ACT) is inflight
                kv_view = cc_out[:, :].rearrange("r (p f) -> p r f", p=128)
                kv_kp = wpool.tile([128, 2, 2 * NBINS], BF16, tag="kv_kp")
                kv_vp = wpool.tile([128, 2, 2 * NBINS], BF16, tag="kv_vp")
                nc.sync.dma_start(kv_kp[:], kv_view[:, :, 0:2 * NBINS])
                nc.scalar.dma_start(kv_vp[:], kv_view[:, :, 2 * NBINS:4 * NBINS])
                kp_b = wpool.tile([128, 2 * NBINS], BF16, tag="kp_b")
                vp_b = wpool.tile([128, 2, NBINS], BF16, tag="vp_b")
                with nc.allow_low_precision(reason="pair partial sum in bf16"):
                    nc.vector.tensor_add(kp_b[:], kv_kp[:, 0, :],
                                         kv_kp[:, 1, :])
                    nc.gpsimd.tensor_add(vp_b[:], kv_vp[:, 0, :],
                                         kv_vp[:, 1, :])

                # ======== STAGE 2: attention + fused output ========
                attnUs = [None] * 4

                def sim_block(n):
                    # sps 0/1 rotate the convps slots; 2/3 reuse the dead
                    # pooled-low banks
                    if n < 2:
                        sps = pspool.tile([NBINS, 512], F32, tag="convps",
                                          name=f"sps{n}")
                    else:
                        sps = pspool.tile([NBINS, 512], F32, tag="plps",
                                          bufs=1, name=f"sps{n}")
                    for m in range(2):
                        nc.tensor.matmul(
                            sps[:], kp_b[:, m * NBINS:(m + 1) * NBINS],
                            q_sb[:, m, n * 512:n * 512 + 512],
                            start=(m == 0), stop=(m == 1))
                    # UNNORMALIZED softmax numerator (logits bounded)
                    attnU = wpool.tile([NBINS, 512], BF16, tag="attnU", bufs=4,
                                       name=f"attnU{n}")
                    nc.scalar.activation(attnU[:], sps[:], AF.Exp,
                                         bias=0.0, scale=0.0625)
                    attnUs[n] = attnU

                sim_block(0)
                # W'T = vp^T @ wfused^T while ACT runs exp(block 0)
                WT_ps = pspool.tile([NBINS, OUT_C], F32, tag="convps")
                for m in range(2):
                    nc.tensor.matmul(WT_ps[:], vp_b[:, m, :], wf_sb[:, m, :],
                                     start=(m == 0), stop=(m == 1))
                nc.vector.tensor_copy(WT_sb[:], WT_ps[:])
                sim_block(1)
                sim_block(2)
                sim_block(3)

                # combine split: 10 tiles via one-pass DVE stt, 6 via ACT
                # scale + Pool add. out DMAs: SP 10 / Pool 5 / ACT 1, last
                # block spread across queues.
                dve_cc = {(0, 0), (0, 2), (1, 0), (1, 2), (2, 0),
                          (2, 2), (3, 0), (3, 2)}
                dma_eng = [
                    [nc.sync, nc.sync, nc.sync, nc.sync],
                    [nc.sync, nc.sync, nc.gpsimd, nc.gpsimd],
                    [nc.sync, nc.sync, nc.gpsimd, nc.sync],
                    [nc.sync, nc.gpsimd, nc.scalar, nc.gpsimd],
                ]
                for n in range(4):
                    attnU = attnUs[n]
                    s_ps = pspool.tile([128, 4], F32, tag="kvp", bufs=1,
                                       name=f"sps_row{n}")
                    # tiny sum matmuls FIRST so the reciprocal (and the
                    # combine chain behind it) starts before the context mms
                    for cc in range(4):
                        nc.tensor.matmul(
                            s_ps[:, cc:cc + 1],
                            attnU[:, cc * 128:(cc + 1) * 128],
                            ones_col[:], start=True, stop=True)
                    rcol = vpool.tile([128, 4], F32, tag="rcol", bufs=2)
                    nc.vector.reciprocal(rcol[:], s_ps[:])
                    ops_l = [None] * 4
                    for cc in range(4):
                        ops = pspool.tile([128, OUT_C], F32, tag="oconv",
                                          bufs=3)
                        nc.tensor.matmul(
                            ops[:], attnU[:, cc * 128:(cc + 1) * 128],
                            WT_sb[:], start=True, stop=True)
                        ops_l[cc] = ops
                    for cc in range(4):
                        c = n * 4 + cc
                        osb = opool.tile([128, OUT_C], F32, tag="osb")
                        if (n, cc) in dve_cc:
                            # DVE: (ctx * 1/s) + high2 in one pass
                            nc.vector.scalar_tensor_tensor(
                                osb[:], ops_l[cc][:], rcol[:, cc:cc + 1],
                                high2[:, c, :],
                                op0=ALU.mult, op1=ALU.add)
                        else:
                            # ACT scale, then Pool adds the high half
                            ot = opool.tile([128, OUT_C], F32, tag="ot")
                            nc.scalar.activation(
                                ot[:], ops_l[cc][:], AF.Identity,
                                bias=0.0, scale=rcol[:, cc:cc + 1])
                            nc.gpsimd.tensor_add(
                                osb[:], ot[:], high2[:, c, :])
                        dma_eng[n][cc].dma_start(out[:, c, :], osb[:])

    nc.finalize()
    return nc


# ======================= host-side data prep =======================

def _pool_matrix_rows(r0, nrows):
    P = np.zeros((nrows * W, NBINS), np.float32)
    col = 0
    for s in PSP_SIZES:
        hs = np.floor(np.arange(s) * H / s).astype(int)
        he = np.ceil((np.arange(s) + 1) * H / s).astype(int)
        ws = np.floor(np.arange(s) * W / s).astype(int)
        we = np.ceil((np.arange(s) + 1) * W / s).astype(int)
        for bh in range(s):
            for bw in range(s):
                area = (he[bh] - hs[bh]) * (we[bw] - ws[bw])
                m = np.zeros((nrows, W), np.float32)
                lo = max(hs[bh] - r0, 0)
                hi = min(he[bh] - r0, nrows)
                if lo < hi:
                    m[lo:hi, ws[bw]:we[bw]] = 1.0 / area
                P[:, col] = m.reshape(-1)
                col += 1
    return P


def _fold_bn(gamma, beta, mean, var, conv_bias):
    inv = np.asarray(gamma, np.float64) / np.sqrt(np.asarray(var, np.float64) + EPS)
    shift = inv * np.asarray(conv_bias, np.float64) + np.asarray(beta, np.float64) \
        - np.asarray(mean, np.float64) * inv
    return inv.astype(np.float32), shift.astype(np.float32)


def _as_kxm(wt, n_k):
    M, K = wt.shape
    r = np.asarray(wt, np.float32).T.reshape(n_k, 128, M).transpose(1, 0, 2)
    return np.ascontiguousarray(r.astype(ml_dtypes.bfloat16))


def prep_in_maps(inputs):
    bf = ml_dtypes.bfloat16
    lf = np.asarray(inputs['low_feats'], np.float32)
    hf = np.asarray(inputs['high_feats'], np.float32)

    ks, kb = _fold_bn(inputs['k_gamma'], inputs['k_beta'], inputs['k_mean'],
                      inputs['k_var'], inputs['bk'])
    qs, qb = _fold_bn(inputs['q_gamma'], inputs['q_beta'], inputs['q_mean'],
                      inputs['q_var'], inputs['bq'])
    os_, ob = _fold_bn(inputs['o_gamma'], inputs['o_beta'], inputs['o_mean'],
                       inputs['o_var'], inputs['bo'])

    # conv_k weights with the k-BN scale folded in; the k bias is added
    # on-device as a rank-1 matmul
    wk_f = np.asarray(inputs['wk'], np.float32) * ks[:, None]
    wk_h = _as_kxm(wk_f, 8)
    wv_h = _as_kxm(np.asarray(inputs['wv'], np.float32), 8)
    kbbc = np.ascontiguousarray(
        np.broadcast_to(np.asarray(kb, np.float32)[None, :], (128, KEY_C)))
    bvrow = np.asarray(inputs['bv'], np.float32).reshape(1, VAL_C).astype(bf)

    wq_h = _as_kxm(np.asarray(inputs['wq']), 4)
    qvec = np.stack([qs[0:128], qs[128:256], qb[0:128], qb[128:256]],
                    1).astype(np.float32)

    # fused context path: W-conv folded into conv_o's ctx half + o-BN scale
    wo = np.asarray(inputs['wo'], np.float32)
    wW = np.asarray(inputs['wW'], np.float32)
    bW = np.asarray(inputs['bW'], np.float32)
    wfused = os_[:, None] * (wo[:, :OUT_C] @ wW)          # [512, 256]
    wo_high = os_[:, None] * wo[:, OUT_C:]                # [512, 512]
    obias_t = ob + os_ * (wo[:, :OUT_C] @ bW)             # [512]
    wf_h = _as_kxm(wfused, 2)
    wo_h = _as_kxm(wo_high, 4)
    obrow = np.ascontiguousarray(
        np.broadcast_to(obias_t[None, :], (128, OUT_C)).astype(np.float32))

    in_maps = []
    for core in range(N_CORES):
        b, half = core // 2, core % 2
        r0 = half * 32
        xl = lf[b, :, r0:r0 + 32, :].reshape(LOW_C, PX)
        xl_h = np.ascontiguousarray(
            xl.reshape(8, 128, PX).transpose(1, 0, 2).astype(bf))
        xlt_h = np.ascontiguousarray(
            xl.T.reshape(NCHUNK, 128, LOW_C).transpose(1, 0, 2).astype(bf))
        xh = hf[b, :, r0:r0 + 32, :].reshape(HIGH_C, PX)
        xh_h = np.ascontiguousarray(
            xh.reshape(4, 128, PX).transpose(1, 0, 2).astype(bf))
        P = _pool_matrix_rows(r0, 32)
        P_h = np.ascontiguousarray(
            P.reshape(16, 128, NBINS).transpose(1, 0, 2).astype(bf))
        csum = P.sum(0).reshape(1, NBINS).astype(bf)
        in_maps.append({
            "xlow": xl_h, "xlt": xlt_h, "xhigh": xh_h,
            "wkT": wk_h, "wvT": wv_h, "wqT": wq_h, "woT": wo_h, "wfT": wf_h,
            "Pmat": P_h,
            "kbbc": kbbc, "bvrow": bvrow, "csum": csum,
            "qvec": qvec, "obrow": obrow,
        })
    return in_maps


def assemble_output(core_outs):
    full = np.zeros((B, OUT_C, H, W), np.float32)
    for core in range(N_CORES):
        b, half = core // 2, core % 2
        r0 = half * 32
        arr = np.asarray(core_outs[core]["out"])  # [128, 16, 512] px-major
        px_oc = arr.transpose(1, 0, 2).reshape(PX, OUT_C)
        full[b, :, r0:r0 + 32, :] = px_oc.T.reshape(OUT_C, 32, W)
    return full


_CACHED_NC = None


def kernel(**inputs) -> np.ndarray:
    global _CACHED_NC
    if _CACHED_NC is None:
        _CACHED_NC = build_kernel(n_rep=1)
    in_maps = prep_in_maps(inputs)
    res = bass_utils.run_bass_kernel_spmd(
        _CACHED_NC, in_maps, core_ids=list(range(N_CORES)))
    return assemble_output(res.results)


# revision 16
# speedup vs baseline: 1.5142x; 1.0327x over previous
"""AFNB (Asymmetric Fusion Non-local Block) — distributed Bass kernel for
8 Trainium2 NeuronCores. Self-contained: builds the Bass/Tile graph, shards
the full inputs, runs SPMD via bass_utils.run_bass_kernel_spmd, and gathers
the full output.

Sharding: data-parallel over (batch, row-half) -> 8 shards of 2048 pixels
(batch b = core//2, rows r0 = (core%2)*32 .. +32).

v4 design (vs the v3 baseline at ~90us sim / ~93us HW):
- The value path exploits linearity of PSP pooling: PSP(conv_v(x)) =
  wv @ PSP(x). The input xlow is pooled directly on the PE from a px-major
  copy (xlt, shipped as an extra input; host prep is free), then vp =
  wvT @ pooled_low costs 1.8k cycles. This removes the full-res conv_v
  (-33k PE cycles) at the cost of 14k pooling cycles that double as
  PE-ramp work.
- The kp|vp AllGather (fixed ~15us launch + transfer) is issued right
  after stage 1 (~27us, vs ~44us in v3) and hidden under conv_q and the
  px-major conv_o-high, whose 49k cycles of PE work almost exactly cover
  the ~20.6us collective.
- Softmax normalization is deferred past the output matmul: the context
  matmuls consume the UNNORMALIZED exp map in px-major orientation
  (lhsT = attnU chunk), so out[px, oc]*(1/s[px]) is a per-partition scale
  applied by ACT/DVE in the final combine, where the high half (+ o-bias,
  folded during the cover phase from a host-prebroadcast row) is added.
  This removes the per-block broadcast matmuls and the serial
  sum->recip->mul chain in front of the context matmuls.

All matmul operands are bf16 (PSUM accumulation fp32). exp() needs no
max-subtraction: |sim|/16 < ~5.
"""
import numpy as np
import ml_dtypes

import concourse.bass as bass
import concourse.mybir as mybir
import concourse.tile as tile
from concourse.bacc import Bacc
from concourse import bass_utils

F32 = mybir.dt.float32
BF16 = mybir.dt.bfloat16
AF = mybir.ActivationFunctionType
ALU = mybir.AluOpType

N_CORES = 8
B, H, W = 4, 64, 64
LOW_C, HIGH_C, KEY_C, VAL_C, OUT_C = 1024, 512, 256, 256, 512
PSP_SIZES = (1, 3, 6, 8)
NBINS = 110
PX = 2048
NCHUNK = 16  # px chunks of 128
EPS = 1e-5

PACK_N = 4 * NBINS * 128  # kp (2x[128,110]) + vp (2x[128,110]) partials


def build_kernel(n_rep: int = 1):
    nc = Bacc("TRN2", target_bir_lowering=False, num_devices=N_CORES)

    xlow = nc.dram_tensor("xlow", [128, 8, PX], BF16, kind="ExternalInput")
    xlt = nc.dram_tensor("xlt", [128, NCHUNK, LOW_C], BF16,
                         kind="ExternalInput")
    xhigh = nc.dram_tensor("xhigh", [128, 4, PX], BF16, kind="ExternalInput")
    wkT = nc.dram_tensor("wkT", [128, 8, KEY_C], BF16, kind="ExternalInput")
    wvT = nc.dram_tensor("wvT", [128, 8, VAL_C], BF16, kind="ExternalInput")
    wqT = nc.dram_tensor("wqT", [128, 4, KEY_C], BF16, kind="ExternalInput")
    woT = nc.dram_tensor("woT", [128, 4, OUT_C], BF16, kind="ExternalInput")
    wfT = nc.dram_tensor("wfT", [128, 2, OUT_C], BF16, kind="ExternalInput")
    Pmat = nc.dram_tensor("Pmat", [128, 16, NBINS], BF16, kind="ExternalInput")
    kbbc = nc.dram_tensor("kbbc", [128, KEY_C], F32, kind="ExternalInput")
    bvrow = nc.dram_tensor("bvrow", [1, VAL_C], BF16, kind="ExternalInput")
    csum = nc.dram_tensor("csum", [1, NBINS], BF16, kind="ExternalInput")
    qvec = nc.dram_tensor("qvec", [128, 4], F32, kind="ExternalInput")
    obrow = nc.dram_tensor("obrow", [128, OUT_C], F32, kind="ExternalInput")
    out = nc.dram_tensor("out", [128, NCHUNK, OUT_C], F32,
                         kind="ExternalOutput")

    with tile.TileContext(nc) as tc:
        with (
            tc.tile_pool(name="const", bufs=1) as cpool,
            tc.tile_pool(name="xin", bufs=2) as xpool,
            tc.tile_pool(name="work", bufs=2) as wpool,
            tc.tile_pool(name="vecs", bufs=4) as vpool,
            tc.tile_pool(name="outp", bufs=4) as opool,
            tc.tile_pool(name="psum", bufs=2, space="PSUM") as pspool,
            tc.tile_pool(name="dram", bufs=1, space="DRAM") as dpool,
        ):
            wk_sb = cpool.tile([128, 8, KEY_C], BF16)
            wv_sb = cpool.tile([128, 8, VAL_C], BF16)
            wq_sb = cpool.tile([128, 4, KEY_C], BF16)
            wo_sb = cpool.tile([128, 4, OUT_C], BF16)
            wf_sb = cpool.tile([128, 2, OUT_C], BF16)
            P_sb = cpool.tile([128, 16, NBINS], BF16)
            kb_sb = cpool.tile([128, KEY_C], F32)
            bv_sb = cpool.tile([1, VAL_C], BF16)
            cs_sb = cpool.tile([1, NBINS], BF16)
            qv_sb = cpool.tile([128, 4], F32)
            ob_sb = cpool.tile([128, OUT_C], F32)
            xh_sb = cpool.tile([128, 4, PX], BF16, tag="xhsb")
            q_sb = cpool.tile([128, 2, PX], BF16, tag="qsb")
            high2 = cpool.tile([128, NCHUNK, OUT_C], BF16, tag="hsb")
            WT_sb = cpool.tile([NBINS, OUT_C], BF16, tag="WT")
            pl_sb = cpool.tile([128, 8, NBINS], BF16, tag="plsb")

            for rep in range(n_rep):
                if rep > 0:
                    tc.strict_bb_all_engine_barrier()
                # warm tiles via memset: no DMA dependency for PE ramp work
                warm = vpool.tile([128, 128], BF16, tag="warm")
                nc.vector.memset(warm[:], 0.0)
                warm2 = vpool.tile([128, 1], F32, tag="warm2")
                # ACT warmup: hoist the act-table load off the critical path
                nc.scalar.activation(warm2[:], warm[:, 0:1], AF.Relu)
                nc.scalar.activation(warm2[:], warm[:, 0:1], AF.Exp)
                ones_col = cpool.tile([NBINS, 1], BF16, tag="ones_col")
                nc.vector.memset(ones_col[:], 1.0)

                warm_ps = pspool.tile([128, 512], F32, tag="convps")
                for _w in range(20):
                    nc.tensor.matmul(warm_ps[:, 0:128], warm[:], warm[:],
                                     start=True, stop=True)

                # ---- DMA schedule ----
                # ACT queue: small rows only (ACT must stay free for the
                # k-relus that gate the kpool matmuls)
                xlt_blks = [None] * NCHUNK
                nc.scalar.dma_start(kb_sb[:], kbbc[:])
                nc.scalar.dma_start(bv_sb[:], bvrow[:])
                nc.scalar.dma_start(cs_sb[:], csum[:])
                for c in range(NCHUNK):
                    xlt_blks[c] = xpool.tile([128, LOW_C], BF16, tag="xlt",
                                             bufs=16, name=f"xlt{c}")
                # Pool queue: P first (gates lowpool), all xlt chunks, then
                # cover weights
                nc.gpsimd.dma_start(P_sb[:], Pmat[:])
                for c in range(NCHUNK):
                    nc.gpsimd.dma_start(xlt_blks[c][:], xlt[:, c, :])
                nc.gpsimd.dma_start(wq_sb[:], wqT[:])
                nc.gpsimd.dma_start(qv_sb[:], qvec[:])
                nc.gpsimd.dma_start(wo_sb[:], woT[:])
                nc.gpsimd.dma_start(wf_sb[:], wfT[:])
                nc.gpsimd.dma_start(ob_sb[:], obrow[:])
                # SP queue: conv_k weights + ch-major xlow stream + xh
                xl_blks = [xpool.tile([128, 8, 512], BF16, tag="xlblk",
                                      bufs=4, name=f"xlblk{n}")
                           for n in range(4)]
                nc.sync.dma_start(wk_sb[:], wkT[:])
                for n in range(4):
                    nc.sync.dma_start(xl_blks[n][:, 0:4, :],
                                      xlow[:, 0:4, 512 * n:512 * n + 512])
                    nc.sync.dma_start(xl_blks[n][:, 4:8, :],
                                      xlow[:, 4:8, 512 * n:512 * n + 512])
                nc.sync.dma_start(wv_sb[:], wvT[:])
                nc.sync.dma_start(xh_sb[:], xhigh[:])

                # ==== STAGE 1 ====
                # long-lived PSUM accumulators:
                #   pl_ps: pooled xlow, 8 groups of [128, 110]
                #   kvp_ps: kp (2 groups) + vp (2 groups) in one bank
                # groups padded to 128 cols so no [*,110] group straddles
                # a PSUM bank boundary
                pl_ps = pspool.tile([128, 8, 128], F32, bufs=1, tag="plps")
                kvp_ps = pspool.tile([128, 4, NBINS], F32, bufs=1, tag="kvp")

                # vp bias bv x colsum(P_local) opens the vp groups
                for m in range(2):
                    nc.tensor.matmul(
                        kvp_ps[:, 2 + m, :], bv_sb[:, m * 128:(m + 1) * 128],
                        cs_sb[:], start=True, stop=False,
                        skip_group_check=True)

                def lowpool_mms(c):
                    for b in range(8):
                        nc.tensor.matmul(
                            pl_ps[:, b, 0:NBINS],
                            xlt_blks[c][:, b * 128:(b + 1) * 128],
                            P_sb[:, c, :],
                            start=(c == 0), stop=(c == NCHUNK - 1),
                            skip_group_check=True)

                # pool the first chunks while xlow/wk stream in
                for c in range(6):
                    lowpool_mms(c)

                # fused conv_k px-major + PSP pooling of relu(k); the pool
                # matmuls of chunk c-1 issue between the conv matmuls of
                # chunk c so the PE never waits on the ACT post-processing
                ksbs = [None] * NCHUNK

                def kpool_mms(c):
                    for m in range(2):
                        nc.tensor.matmul(
                            kvp_ps[:, m, :],
                            ksbs[c][:, m * 128:(m + 1) * 128],
                            P_sb[:, c, :],
                            start=(c == 0), stop=(c == NCHUNK - 1),
                            skip_group_check=True)

                lp_next = 6
                for c in range(NCHUNK):
                    n, cc = c // 4, c % 4
                    kps = pspool.tile([128, 512], F32, tag="convps",
                                      bufs=2)
                    for kc in range(8):
                        nc.tensor.matmul(
                            kps[:, 0:KEY_C],
                            xl_blks[n][:, kc, cc * 128:cc * 128 + 128],
                            wk_sb[:, kc, :],
                            start=(kc == 0), stop=(kc == 7),
                            skip_group_check=True)
                        if kc == 4 and c > 0:
                            kpool_mms(c - 1)
                    # k-BN bias (prebroadcast row) on DVE, then ReLU on ACT;
                    # keeps the bias matmul off the pre-collective PE path
                    kraw = wpool.tile([128, KEY_C], BF16, tag="kraw", bufs=4)
                    nc.vector.tensor_add(kraw[:], kps[:, 0:KEY_C], kb_sb[:])
                    ksb = wpool.tile([128, KEY_C], BF16, tag="ksb", bufs=4)
                    nc.scalar.activation(ksb[:], kraw[:], AF.Relu)
                    ksbs[c] = ksb
                    if lp_next < NCHUNK:
                        lowpool_mms(lp_next)
                        lp_next += 1
                    if lp_next == NCHUNK:
                        # all pooled-low groups closed: copy out for vp
                        for b in range(8):
                            nc.vector.tensor_copy(pl_sb[:, b, :],
                                                  pl_ps[:, b, 0:NBINS])
                        lp_next += 1
                    if c == 13:
                        # vp = wvT @ pooled_low onto the bias groups; off the
                        # pack gate (kpool 15) critical path
                        for b in range(8):
                            for m in range(2):
                                nc.tensor.matmul(
                                    kvp_ps[:, 2 + m, :],
                                    wv_sb[:, b, m * 128:(m + 1) * 128],
                                    pl_sb[:, b, :],
                                    start=False, stop=(b == 7),
                                    skip_group_check=True)
                kpool_mms(NCHUNK - 1)

                # pack partials; single pairwise AllGather
                kv_h = wpool.tile([128, 4 * NBINS], BF16, tag="kv_h")
                nc.vector.tensor_copy(kv_h[:], kvp_ps[:])
                cc_in = dpool.tile([PACK_N], BF16, tag=f"ccin_{rep}")
                cc_out = dpool.tile([2, PACK_N], BF16, tag=f"ccout_{rep}")
                nc.sync.dma_start(
                    cc_in[:].rearrange("(p f) -> p f", p=128), kv_h[:])
                nc.gpsimd.collective_compute(
                    "AllGather", mybir.AluOpType.bypass,
                    replica_groups=[[0, 1], [2, 3], [4, 5], [6, 7]],
                    ins=[cc_in[:].opt()], outs=[cc_out[:].opt()])

                # ==== COVER PHASE (hides the collective) ====
                # conv_q — k-major: lhsT = wq, rhs = xh
                for n in range(4):
                    for m in range(2):
                        qps = pspool.tile([128, 512], F32, tag="convps")
                        for kc in range(4):
                            nc.tensor.matmul(
                                qps[:], wq_sb[:, kc, m * 128:(m + 1) * 128],
                                xh_sb[:, kc, n * 512:(n + 1) * 512],
                                start=(kc == 0), stop=(kc == 3))
                        nc.scalar.activation(
                            q_sb[:, m, n * 512:(n + 1) * 512], qps[:], AF.Relu,
                            bias=qv_sb[:, 2 + m:3 + m], scale=qv_sb[:, m:m + 1])

                # conv_o high half, px-major: lhsT = xh chunk, rhs = wo.
                # high2 = conv + o-bias (prebroadcast row), parked bf16.
                for c in range(NCHUNK):
                    hps = pspool.tile([128, OUT_C], F32, tag="oconv", bufs=3)
                    for kc in range(4):
                        nc.tensor.matmul(
                            hps[:], xh_sb[:, kc, c * 128:c * 128 + 128],
                            wo_sb[:, kc, :],
                            start=(kc == 0), stop=(kc == 3))
                    nc.vector.tensor_add(high2[:, c, :], hps[:], ob_sb[:])

                # PE keep-warm fill through the collective window: the PE
                # would otherwise idle ~7us here and drop its p-state, which
                # would halve the clock for the first ~3us of stage 2
                warmf = pspool.tile([128, 512], F32, tag="convps",
                                    name="warmf")
                for _w in range(72):
                    nc.tensor.matmul(warmf[:, 0:128], warm[:], warm[:],
                                     start=True, stop=True)

                # combine gathered partials: kp half lands first (on SP) so
                # the sim matmuls can start while the vp half (ACT) is inflight
                kv_view = cc_out[:, :].rearrange("r (p f) -> p r f", p=128)
                kv_kp = wpool.tile([128, 2, 2 * NBINS], BF16, tag="kv_kp")
                kv_vp = wpool.tile([128, 2, 2 * NBINS], BF16, tag="kv_vp")
                nc.sync.dma_start(kv_kp[:], kv_view[:, :, 0:2 * NBINS])
                nc.scalar.dma_start(kv_vp[:], kv_view[:, :, 2 * NBINS:4 * NBINS])
                kp_b = wpool.tile([128, 2 * NBINS], BF16, tag="kp_b")
                vp_b = wpool.tile([128, 2, NBINS], BF16, tag="vp_b")
                with nc.allow_low_precision(reason="pair partial sum in bf16"):
                    nc.vector.tensor_add(kp_b[:], kv_kp[:, 0, :],
                                         kv_kp[:, 1, :])
                    nc.gpsimd.tensor_add(vp_b[:], kv_vp[:, 0, :],
                                         kv_vp[:, 1, :])

                # ======== STAGE 2: attention + fused output ========
                attnUs = [None] * 4

                def sim_block(n):
                    # sps 0/1 rotate the convps slots; 2/3 reuse the dead
                    # pooled-low banks
                    if n < 2:
                        sps = pspool.tile([NBINS, 512], F32, tag="convps",
                                          name=f"sps{n}")
                    else:
                        sps = pspool.tile([NBINS, 512], F32, tag="plps",
                                          bufs=1, name=f"sps{n}")
                    for m in range(2):
                        nc.tensor.matmul(
                            sps[:], kp_b[:, m * NBINS:(m + 1) * NBINS],
                            q_sb[:, m, n * 512:n * 512 + 512],
                            start=(m == 0), stop=(m == 1))
                    # UNNORMALIZED softmax numerator (logits bounded)
                    attnU = wpool.tile([NBINS, 512], BF16, tag="attnU", bufs=4,
                                       name=f"attnU{n}")
                    nc.scalar.activation(attnU[:], sps[:], AF.Exp,
                                         bias=0.0, scale=0.0625)
                    attnUs[n] = attnU

                sim_block(0)
                # W'T = vp^T @ wfused^T while ACT runs exp(block 0)
                WT_ps = pspool.tile([NBINS, OUT_C], F32, tag="convps")
                for m in range(2):
                    nc.tensor.matmul(WT_ps[:], vp_b[:, m, :], wf_sb[:, m, :],
                                     start=(m == 0), stop=(m == 1))
                nc.vector.tensor_copy(WT_sb[:], WT_ps[:])
                sim_block(1)
                sim_block(2)
                sim_block(3)

                # combine split: 10 tiles via one-pass DVE stt, 6 via ACT
                # scale + Pool add. out DMAs: SP 10 / Pool 5 / ACT 1, last
                # block spread across queues.
                dve_cc = {(0, 0), (0, 2), (1, 0), (1, 2), (2, 0),
                          (2, 2), (3, 0), (3, 2)}
                dma_eng = [
                    [nc.sync, nc.sync, nc.sync, nc.sync],
                    [nc.sync, nc.sync, nc.gpsimd, nc.gpsimd],
                    [nc.sync, nc.sync, nc.gpsimd, nc.sync],
                    [nc.sync, nc.gpsimd, nc.scalar, nc.gpsimd],
                ]
                for n in range(4):
                    attnU = attnUs[n]
                    s_ps = pspool.tile([128, 4], F32, tag="kvp", bufs=1,
                                       name=f"sps_row{n}")
                    # tiny sum matmuls FIRST so the reciprocal (and the
                    # combine chain behind it) starts before the context mms
                    for cc in range(4):
                        nc.tensor.matmul(
                            s_ps[:, cc:cc + 1],
                            attnU[:, cc * 128:(cc + 1) * 128],
                            ones_col[:], start=True, stop=True)
                    rcol = vpool.tile([128, 4], F32, tag="rcol", bufs=2)
                    nc.vector.reciprocal(rcol[:], s_ps[:])
                    ops_l = [None] * 4
                    for cc in range(4):
                        ops = pspool.tile([128, OUT_C], F32, tag="oconv",
                                          bufs=3)
                        nc.tensor.matmul(
                            ops[:], attnU[:, cc * 128:(cc + 1) * 128],
                            WT_sb[:], start=True, stop=True)
                        ops_l[cc] = ops
                    for cc in range(4):
                        c = n * 4 + cc
                        osb = opool.tile([128, OUT_C], F32, tag="osb")
                        if (n, cc) in dve_cc:
                            # DVE: (ctx * 1/s) + high2 in one pass
                            nc.vector.scalar_tensor_tensor(
                                osb[:], ops_l[cc][:], rcol[:, cc:cc + 1],
                                high2[:, c, :],
                                op0=ALU.mult, op1=ALU.add)
                        else:
                            # ACT scale, then Pool adds the high half
                            ot = opool.tile([128, OUT_C], F32, tag="ot")
                            nc.scalar.activation(
                                ot[:], ops_l[cc][:], AF.Identity,
                                bias=0.0, scale=rcol[:, cc:cc + 1])
                            nc.gpsimd.tensor_add(
                                osb[:], ot[:], high2[:, c, :])
                        dma_eng[n][cc].dma_start(out[:, c, :], osb[:])

    nc.finalize()
    return nc


# ======================= host-side data prep =======================

def _pool_matrix_rows(r0, nrows):
    P = np.zeros((nrows * W, NBINS), np.float32)
    col = 0
    for s in PSP_SIZES:
        hs = np.floor(np.arange(s) * H / s).astype(int)
        he = np.ceil((np.arange(s) + 1) * H / s).astype(int)
        ws = np.floor(np.arange(s) * W / s).astype(int)
        we = np.ceil((np.arange(s) + 1) * W / s).astype(int)
        for bh in range(s):
            for bw in range(s):
                area = (he[bh] - hs[bh]) * (we[bw] - ws[bw])
                m = np.zeros((nrows, W), np.float32)
                lo = max(hs[bh] - r0, 0)
                hi = min(he[bh] - r0, nrows)
                if lo < hi:
                    m[lo:hi, ws[bw]:we[bw]] = 1.0 / area
                P[:, col] = m.reshape(-1)
                col += 1
    return P


def _fold_bn(gamma, beta, mean, var, conv_bias):
    inv = np.asarray(gamma, np.float64) / np.sqrt(np.asarray(var, np.float64) + EPS)
    shift = inv * np.asarray(conv_bias, np.float64) + np.asarray(beta, np.float64) \
        - np.asarray(mean, np.float64) * inv
    return inv.astype(np.float32), shift.astype(np.float32)


def _as_kxm(wt, n_k):
    M, K = wt.shape
    r = np.asarray(wt, np.float32).T.reshape(n_k, 128, M).transpose(1, 0, 2)
    return np.ascontiguousarray(r.astype(ml_dtypes.bfloat16))


def prep_in_maps(inputs):
    bf = ml_dtypes.bfloat16
    lf = np.asarray(inputs['low_feats'], np.float32)
    hf = np.asarray(inputs['high_feats'], np.float32)

    ks, kb = _fold_bn(inputs['k_gamma'], inputs['k_beta'], inputs['k_mean'],
                      inputs['k_var'], inputs['bk'])
    qs, qb = _fold_bn(inputs['q_gamma'], inputs['q_beta'], inputs['q_mean'],
                      inputs['q_var'], inputs['bq'])
    os_, ob = _fold_bn(inputs['o_gamma'], inputs['o_beta'], inputs['o_mean'],
                       inputs['o_var'], inputs['bo'])

    # conv_k weights with the k-BN scale folded in; the k bias is added
    # on-device as a rank-1 matmul
    wk_f = np.asarray(inputs['wk'], np.float32) * ks[:, None]
    wk_h = _as_kxm(wk_f, 8)
    wv_h = _as_kxm(np.asarray(inputs['wv'], np.float32), 8)
    kbbc = np.ascontiguousarray(
        np.broadcast_to(np.asarray(kb, np.float32)[None, :], (128, KEY_C)))
    bvrow = np.asarray(inputs['bv'], np.float32).reshape(1, VAL_C).astype(bf)

    wq_h = _as_kxm(np.asarray(inputs['wq']), 4)
    qvec = np.stack([qs[0:128], qs[128:256], qb[0:128], qb[128:256]],
                    1).astype(np.float32)

    # fused context path: W-conv folded into conv_o's ctx half + o-BN scale
    wo = np.asarray(inputs['wo'], np.float32)
    wW = np.asarray(inputs['wW'], np.float32)
    bW = np.asarray(inputs['bW'], np.float32)
    wfused = os_[:, None] * (wo[:, :OUT_C] @ wW)          # [512, 256]
    wo_high = os_[:, None] * wo[:, OUT_C:]                # [512, 512]
    obias_t = ob + os_ * (wo[:, :OUT_C] @ bW)             # [512]
    wf_h = _as_kxm(wfused, 2)
    wo_h = _as_kxm(wo_high, 4)
    obrow = np.ascontiguousarray(
        np.broadcast_to(obias_t[None, :], (128, OUT_C)).astype(np.float32))

    in_maps = []
    for core in range(N_CORES):
        b, half = core // 2, core % 2
        r0 = half * 32
        xl = lf[b, :, r0:r0 + 32, :].reshape(LOW_C, PX)
        xl_h = np.ascontiguousarray(
            xl.reshape(8, 128, PX).transpose(1, 0, 2).astype(bf))
        xlt_h = np.ascontiguousarray(
            xl.T.reshape(NCHUNK, 128, LOW_C).transpose(1, 0, 2).astype(bf))
        xh = hf[b, :, r0:r0 + 32, :].reshape(HIGH_C, PX)
        xh_h = np.ascontiguousarray(
            xh.reshape(4, 128, PX).transpose(1, 0, 2).astype(bf))
        P = _pool_matrix_rows(r0, 32)
        P_h = np.ascontiguousarray(
            P.reshape(16, 128, NBINS).transpose(1, 0, 2).astype(bf))
        csum = P.sum(0).reshape(1, NBINS).astype(bf)
        in_maps.append({
            "xlow": xl_h, "xlt": xlt_h, "xhigh": xh_h,
            "wkT": wk_h, "wvT": wv_h, "wqT": wq_h, "woT": wo_h, "wfT": wf_h,
            "Pmat": P_h,
            "kbbc": kbbc, "bvrow": bvrow, "csum": csum,
            "qvec": qvec, "obrow": obrow,
        })
    return in_maps


def assemble_output(core_outs):
    full = np.zeros((B, OUT_C, H, W), np.float32)
    for core in range(N_CORES):
        b, half = core // 2, core % 2
        r0 = half * 32
        arr = np.asarray(core_outs[core]["out"])  # [128, 16, 512] px-major
        px_oc = arr.transpose(1, 0, 2).reshape(PX, OUT_C)
        full[b, :, r0:r0 + 32, :] = px_oc.T.reshape(OUT_C, 32, W)
    return full


_CACHED_NC = None


def kernel(**inputs) -> np.ndarray:
    global _CACHED_NC
    if _CACHED_NC is None:
        _CACHED_NC = build_kernel(n_rep=1)
    in_maps = prep_in_maps(inputs)
    res = bass_utils.run_bass_kernel_spmd(
        _CACHED_NC, in_maps, core_ids=list(range(N_CORES)))
    return assemble_output(res.results)
